# revision 6
# baseline (speedup 1.0000x reference)
"""Trainium2 Bass kernel for the regularized newsvendor layer.

Per batch row p (weights over sorted support y), find the root z* of
  f(z) = sum_i p_i*[(y_i<=z) - CF] + g(z),
  g(z) = GAMMA * sqrt(B) * t / sqrt(t^2 + E0),  t = z - A/B,
  S = sum p, B = sum p^2, A = sum p^2 y, C = sum p^2 y^2,
  E0 = (C - A^2/B + 1e-12)/B,
then emit (z, pinball(y-z), y-z, p*(y-z)).

Instead of the reference's 60-step bisection (60 full passes over the
support), f's zero is located directly: it either sits exactly on a support
point y[k*] (the step part crosses 0 there) or between support points where
the smooth part solves in closed form:
  z = A/B - u*sqrt(E0/(GAMMA^2*B - u^2)),  u = c_{k*-1} - CF*S.
Both cases collapse to z = clamp(z_closed, 0, min(y[k*], 1)).
k* is found hierarchically: prefix sums at 16 chunk boundaries (one chunked
reduce + a 16-wide hardware scan), then an indirect-DMA gather of the one
128-wide chunk containing the crossing, resolved exactly with a scan seeded
by the chunk prefix.

Engine notes: this environment cannot run ScalarEngine activations or the
custom TensorTensorReduce (both wedge the exec unit), so everything is DVE /
GPSIMD / PE / DMA only; rsqrt and sqrt use the int32 bit-trick seed plus
Newton steps on the vector engine.

Sharding: pure data parallel across 8 cores, 512 batch rows each; y
replicated. Full inputs in, full outputs out.
"""

import sys
import functools

import numpy as np

sys.path.insert(0, "/opt/trn_rl_repo")

import concourse.bass as bass
import concourse.bacc as bacc
import concourse.mybir as mybir
import concourse.tile as tile
from concourse.bass_utils import run_bass_kernel_spmd

CF = 0.7
GAMMA = 0.1
N = 2048
BATCH = 4096
NCORES = 8
R = BATCH // NCORES   # rows per core = 512
P = 128
NT = R // P           # tiles per core = 4
NCH = 16              # chunks per row
L = N // NCH          # chunk length = 128

F32 = mybir.dt.float32
I32 = mybir.dt.int32
OP = mybir.AluOpType
AF = mybir.ActivationFunctionType
MAGIC = 0x5F3759DF


def build_nc(rows=R):
    nt = rows // P
    w = nt * NCH      # packed chunk width (64)
    wl = nt * L       # packed window width (512)
    nc = bacc.Bacc(None, target_bir_lowering=False)

    x_d = nc.dram_tensor("x", [rows, N], F32, kind="ExternalInput")
    y_d = nc.dram_tensor("y", [N], F32, kind="ExternalInput")
    z_d = nc.dram_tensor("z", [rows, 1], F32, kind="ExternalOutput")
    err_d = nc.dram_tensor("err", [rows, N], F32, kind="ExternalOutput")
    pin_d = nc.dram_tensor("pin", [rows, N], F32, kind="ExternalOutput")
    werr_d = nc.dram_tensor("werr", [rows, N], F32, kind="ExternalOutput")

    x_chunks = x_d.rearrange("r (c l) -> (r c) l", l=L)   # [rows*16, 128]
    y_chunks = y_d.rearrange("(c l) -> c l", l=L)         # [16, 128]

    def bc(t4, inner):
        # [P, nt] per-tile scalars -> broadcast view [P, nt, inner]
        return t4[:, :, None].broadcast_to([P, nt, inner])

    with tile.TileContext(nc) as tc:
        with (
            tc.tile_pool(name="const", bufs=1) as cpool,
            tc.tile_pool(name="state", bufs=1) as stp,
            tc.tile_pool(name="xp", bufs=nt) as xpool,
            tc.tile_pool(name="big", bufs=2) as bpool,
            tc.tile_pool(name="sml", bufs=2) as spool,
        ):
            # ---------- constants ----------
            yb = cpool.tile([P, N], F32)
            nc.sync.dma_start(yb[:], y_d[None, :].broadcast_to([P, N]))
            c15 = cpool.tile([P, wl], F32)
            nc.vector.memset(c15[:], 1.5)
            # gather base offsets: idx0[p, t] = 16*p + 2048*t (int32 -> f32)
            pfi = cpool.tile([P, nt], I32)
            nc.gpsimd.iota(pfi[:], pattern=[[P * NCH, nt]], base=0,
                           channel_multiplier=NCH)
            pft = cpool.tile([P, nt], F32)
            nc.vector.tensor_copy(pft[:], pfi[:])

            def newton(rt, s2t, iters, width):
                # rt <- rsqrt(s2t) via bit-trick seed + Newton iterations
                ii = rt[:].bitcast(I32)
                nc.vector.tensor_scalar(ii, s2t[:].bitcast(I32), 1, None,
                                        op0=OP.logical_shift_right)
                nc.vector.tensor_scalar(ii, ii, -1, None, op0=OP.mult)
                nc.vector.tensor_scalar(ii, ii, MAGIC, None, op0=OP.add)
                r2 = spool.tile([P, width], F32, tag=f"nw_r2_{width}")
                a = spool.tile([P, width], F32, tag=f"nw_a_{width}")
                for _ in range(iters):
                    nc.vector.tensor_tensor(r2[:], rt[:], rt[:], op=OP.mult)
                    nc.vector.tensor_tensor(a[:], s2t[:], r2[:], op=OP.mult)
                    nc.vector.scalar_tensor_tensor(a[:], a[:], -0.5,
                                                   c15[:, 0:width],
                                                   op0=OP.mult, op1=OP.add)
                    nc.vector.tensor_tensor(rt[:], rt[:], a[:], op=OP.mult)

            # ---------- packed state ----------
            B4 = stp.tile([P, nt], F32)
            A4 = stp.tile([P, nt], F32)
            C4 = stp.tile([P, nt], F32)
            cs64 = stp.tile([P, w], F32)
            cb64 = stp.tile([P, w], F32)
            xg = stp.tile([P, wl], F32)
            yg = stp.tile([P, wl], F32)

            xts = []
            # ---------- phase 1: per-tile streams ----------
            for t in range(nt):
                r0 = t * P
                xt = xpool.tile([P, N], F32, tag="xt")
                xts.append(xt)
                nc.sync.dma_start(xt[:], x_d[r0:r0 + P, :])

                q = bpool.tile([P, N], F32, tag="scr")
                nc.vector.scalar_tensor_tensor(q[:], xt[:], 0.0, xt[:],
                                               op0=OP.bypass, op1=OP.mult,
                                               accum_out=B4[:, t:t + 1])
                qy = bpool.tile([P, N], F32, tag="scr")
                nc.vector.scalar_tensor_tensor(qy[:], q[:], 0.0, yb[:],
                                               op0=OP.bypass, op1=OP.mult,
                                               accum_out=A4[:, t:t + 1])
                qy2 = bpool.tile([P, N], F32, tag="scr")
                nc.vector.scalar_tensor_tensor(qy2[:], qy[:], 0.0, yb[:],
                                               op0=OP.bypass, op1=OP.mult,
                                               accum_out=C4[:, t:t + 1])

                nc.vector.tensor_reduce(
                    cs64[:, t * NCH:(t + 1) * NCH],
                    xt.rearrange("p (c l) -> p c l", c=NCH),
                    axis=mybir.AxisListType.X, op=OP.add)
                nc.vector.tensor_tensor_scan(
                    cb64[:, t * NCH:(t + 1) * NCH],
                    cs64[:, t * NCH:(t + 1) * NCH],
                    cs64[:, t * NCH:(t + 1) * NCH],
                    0.0, op0=OP.add, op1=OP.bypass)

            # ---------- phase 2: packed solve ----------
            S4 = cb64[:, NCH - 1::NCH]  # [P, nt] view: per-tile total sums

            Bg = stp.tile([P, nt], F32)
            nc.vector.tensor_scalar(Bg[:], B4[:], 1e-20, None, op0=OP.max)
            invB = stp.tile([P, nt], F32)
            nc.vector.reciprocal(invB[:], Bg[:])
            AB = stp.tile([P, nt], F32)
            nc.vector.tensor_tensor(AB[:], A4[:], invB[:], op=OP.mult)
            CB = spool.tile([P, nt], F32, tag="CB")
            nc.vector.tensor_tensor(CB[:], C4[:], invB[:], op=OP.mult)
            ABsq = spool.tile([P, nt], F32, tag="ABsq")
            nc.vector.tensor_tensor(ABsq[:], AB[:], AB[:], op=OP.mult)
            E0 = stp.tile([P, nt], F32)
            nc.vector.tensor_tensor(E0[:], CB[:], ABsq[:], op=OP.subtract)
            eps12 = spool.tile([P, nt], F32, tag="eps12")
            nc.vector.tensor_scalar(eps12[:], invB[:], 1e-12, None, op0=OP.mult)
            nc.vector.tensor_tensor(E0[:], E0[:], eps12[:], op=OP.add)
            nc.vector.tensor_scalar(E0[:], E0[:], 1e-30, None, op0=OP.max)
            CFS = stp.tile([P, nt], F32)
            nc.vector.tensor_scalar(CFS[:], S4, CF, None, op0=OP.mult)
            G2B = stp.tile([P, nt], F32)
            nc.vector.tensor_scalar(G2B[:], Bg[:], GAMMA * GAMMA, None,
                                    op0=OP.mult)
            rsB = spool.tile([P, nt], F32, tag="rsB")
            newton(rsB, Bg, 2, nt)
            GB = stp.tile([P, nt], F32)
            nc.vector.tensor_tensor(GB[:], Bg[:], rsB[:], op=OP.mult)
            nc.vector.tensor_scalar(GB[:], GB[:], GAMMA, None, op0=OP.mult)

            # chunk-boundary f: find crossing chunk j* per (row, tile)
            ybnd = yb[:, L - 1::L][:, None, :].broadcast_to([P, nt, NCH])
            t64 = spool.tile([P, w], F32, tag="t64")
            t64v = t64.rearrange("p (t c) -> p t c", t=nt)
            nc.vector.tensor_tensor(t64v, ybnd, bc(AB, NCH), op=OP.subtract)
            s64 = spool.tile([P, w], F32, tag="s64")
            nc.vector.tensor_tensor(s64[:], t64[:], t64[:], op=OP.mult)
            nc.vector.tensor_tensor(s64.rearrange("p (t c) -> p t c", t=nt),
                                    s64.rearrange("p (t c) -> p t c", t=nt),
                                    bc(E0, NCH), op=OP.add)
            rh64 = spool.tile([P, w], F32, tag="rh64")
            newton(rh64, s64, 2, w)
            m64 = spool.tile([P, w], F32, tag="m64")
            nc.vector.tensor_tensor(m64[:], t64[:], rh64[:], op=OP.mult)
            nc.vector.tensor_tensor(m64.rearrange("p (t c) -> p t c", t=nt),
                                    m64.rearrange("p (t c) -> p t c", t=nt),
                                    bc(GB, NCH), op=OP.mult)
            h64 = spool.tile([P, w], F32, tag="h64")
            nc.vector.tensor_tensor(h64[:], m64[:], cb64[:], op=OP.add)
            mask64 = spool.tile([P, w], F32, tag="mask64")
            nc.vector.tensor_tensor(mask64.rearrange("p (t c) -> p t c", t=nt),
                                    h64.rearrange("p (t c) -> p t c", t=nt),
                                    bc(CFS, NCH), op=OP.is_lt)
            jf = stp.tile([P, nt], F32)
            nc.vector.tensor_reduce(jf[:],
                                    mask64.rearrange("p (t c) -> p t c", t=nt),
                                    axis=mybir.AxisListType.X, op=OP.add)
            cm64 = spool.tile([P, w], F32, tag="cm64")
            nc.vector.tensor_tensor(cm64[:], cb64[:], mask64[:], op=OP.mult)
            cpch = stp.tile([P, nt], F32)
            nc.vector.tensor_reduce(cpch[:],
                                    cm64.rearrange("p (t c) -> p t c", t=nt),
                                    axis=mybir.AxisListType.X, op=OP.max)

            # gather the crossing chunk of x and y per (row, tile)
            jmin = stp.tile([P, nt], F32)
            nc.vector.tensor_scalar(jmin[:], jf[:], float(NCH - 1), None,
                                    op0=OP.min)
            gxf = spool.tile([P, nt], F32, tag="gxf")
            nc.vector.tensor_tensor(gxf[:], pft[:], jmin[:], op=OP.add)
            gxi = stp.tile([P, nt], I32)
            nc.vector.tensor_copy(gxi[:], gxf[:])
            gyi = stp.tile([P, nt], I32)
            nc.vector.tensor_copy(gyi[:], jmin[:])
            for t in range(nt):
                nc.gpsimd.indirect_dma_start(
                    xg[:, t * L:(t + 1) * L], None, x_chunks,
                    bass.IndirectOffsetOnAxis(ap=gxi[:, t:t + 1], axis=0))
                nc.gpsimd.indirect_dma_start(
                    yg[:, t * L:(t + 1) * L], None, y_chunks,
                    bass.IndirectOffsetOnAxis(ap=gyi[:, t:t + 1], axis=0))

            # within-chunk exact resolution (packed [P, 512])
            cloc = stp.tile([P, wl], F32)
            for t in range(nt):
                nc.vector.tensor_tensor_scan(
                    cloc[:, t * L:(t + 1) * L], xg[:, t * L:(t + 1) * L],
                    xg[:, t * L:(t + 1) * L], cpch[:, t:t + 1],
                    op0=OP.add, op1=OP.bypass)
            tw = spool.tile([P, wl], F32, tag="tw")
            nc.vector.tensor_tensor(tw.rearrange("p (t c) -> p t c", t=nt),
                                    yg.rearrange("p (t c) -> p t c", t=nt),
                                    bc(AB, L), op=OP.subtract)
            s2w = spool.tile([P, wl], F32, tag="s2w")
            nc.vector.tensor_tensor(s2w[:], tw[:], tw[:], op=OP.mult)
            nc.vector.tensor_tensor(s2w.rearrange("p (t c) -> p t c", t=nt),
                                    s2w.rearrange("p (t c) -> p t c", t=nt),
                                    bc(E0, L), op=OP.add)
            rhw = spool.tile([P, wl], F32, tag="rhw")
            newton(rhw, s2w, 2, wl)
            mw = spool.tile([P, wl], F32, tag="mw")
            nc.vector.tensor_tensor(mw[:], tw[:], rhw[:], op=OP.mult)
            nc.vector.tensor_tensor(mw.rearrange("p (t c) -> p t c", t=nt),
                                    mw.rearrange("p (t c) -> p t c", t=nt),
                                    bc(GB, L), op=OP.mult)
            hw = spool.tile([P, wl], F32, tag="hw")
            nc.vector.tensor_tensor(hw[:], mw[:], cloc[:], op=OP.add)
            maskw = spool.tile([P, wl], F32, tag="maskw")
            nc.vector.tensor_tensor(maskw.rearrange("p (t c) -> p t c", t=nt),
                                    hw.rearrange("p (t c) -> p t c", t=nt),
                                    bc(CFS, L), op=OP.is_lt)
            cmw = spool.tile([P, wl], F32, tag="cmw")
            nc.vector.tensor_tensor(cmw[:], cloc[:], maskw[:], op=OP.mult)
            cprev = stp.tile([P, nt], F32)
            nc.vector.tensor_reduce(cprev[:],
                                    cmw.rearrange("p (t c) -> p t c", t=nt),
                                    axis=mybir.AxisListType.X, op=OP.max)
            nc.vector.tensor_tensor(cprev[:], cprev[:], cpch[:], op=OP.max)
            ymw = spool.tile([P, wl], F32, tag="ymw")
            nc.vector.tensor_tensor(ymw[:], yg[:], maskw[:], op=OP.add)
            ystar = stp.tile([P, nt], F32)
            nc.vector.tensor_reduce(ystar[:],
                                    ymw.rearrange("p (t c) -> p t c", t=nt),
                                    axis=mybir.AxisListType.X, op=OP.min)

            # closed form + clamp -> z
            u = stp.tile([P, nt], F32)
            nc.vector.tensor_tensor(u[:], cprev[:], CFS[:], op=OP.subtract)
            u2 = spool.tile([P, nt], F32, tag="u2")
            nc.vector.tensor_tensor(u2[:], u[:], u[:], op=OP.mult)
            den = spool.tile([P, nt], F32, tag="den")
            nc.vector.tensor_tensor(den[:], G2B[:], u2[:], op=OP.subtract)
            nc.vector.tensor_scalar(den[:], den[:], 1e-20, None, op0=OP.max)
            rden = spool.tile([P, nt], F32, tag="rden")
            nc.vector.reciprocal(rden[:], den[:])
            rat = spool.tile([P, nt], F32, tag="rat")
            nc.vector.tensor_tensor(rat[:], E0[:], rden[:], op=OP.mult)
            rsr = spool.tile([P, nt], F32, tag="rsr")
            newton(rsr, rat, 3, nt)
            s1 = spool.tile([P, nt], F32, tag="s1")
            nc.vector.tensor_tensor(s1[:], rat[:], rsr[:], op=OP.mult)
            us = spool.tile([P, nt], F32, tag="us")
            nc.vector.tensor_tensor(us[:], u[:], s1[:], op=OP.mult)
            z4 = stp.tile([P, nt], F32)
            nc.vector.tensor_tensor(z4[:], AB[:], us[:], op=OP.subtract)
            yc = spool.tile([P, nt], F32, tag="yc")
            nc.vector.tensor_scalar(yc[:], ystar[:], 1.0, None, op0=OP.min)
            nc.vector.tensor_tensor(z4[:], z4[:], yc[:], op=OP.min)
            nc.vector.tensor_scalar(z4[:], z4[:], 0.0, None, op0=OP.max)
            nc.sync.dma_start(
                z_d.rearrange("(t p) o -> p (t o)", p=P), z4[:])

            # ---------- phase 3: big outputs ----------
            for t in range(nt):
                r0 = t * P
                xt = xts[t]
                errt = bpool.tile([P, N], F32, tag="errt")
                nc.vector.tensor_scalar(errt[:], yb[:], z4[:, t:t + 1], None,
                                        op0=OP.subtract)
                nc.sync.dma_start(err_d[r0:r0 + P, :], errt[:])
                a7 = bpool.tile([P, N], F32, tag="a7")
                nc.vector.tensor_scalar(a7[:], errt[:], CF, None, op0=OP.mult)
                b3 = bpool.tile([P, N], F32, tag="b3")
                nc.gpsimd.tensor_scalar(b3[:], errt[:], CF - 1.0, None,
                                        op0=OP.mult)
                pin = bpool.tile([P, N], F32, tag="pin")
                nc.vector.tensor_tensor(pin[:], a7[:], b3[:], op=OP.max)
                nc.sync.dma_start(pin_d[r0:r0 + P, :], pin[:])
                werrt = bpool.tile([P, N], F32, tag="werrt")
                nc.gpsimd.tensor_tensor(werrt[:], xt[:], errt[:], op=OP.mult)
                nc.sync.dma_start(werr_d[r0:r0 + P, :], werrt[:])

    nc.finalize()
    return nc


@functools.lru_cache(maxsize=1)
def _cached_nc():
    return build_nc(R)


def run(x, y_supp, trace=False):
    x = np.ascontiguousarray(x, dtype=np.float32)
    y = np.ascontiguousarray(y_supp, dtype=np.float32)
    assert x.shape == (BATCH, N) and y.shape == (N,)
    nc = _cached_nc()
    in_maps = [{"x": x[i * R:(i + 1) * R], "y": y} for i in range(NCORES)]
    res = run_bass_kernel_spmd(nc, in_maps, core_ids=list(range(NCORES)),
                               trace=trace)
    z = np.concatenate([r["z"] for r in res.results], axis=0)
    err = np.concatenate([r["err"] for r in res.results], axis=0)
    pin = np.concatenate([r["pin"] for r in res.results], axis=0)
    werr = np.concatenate([r["werr"] for r in res.results], axis=0)
    return (z, pin, err, werr), res


def kernel(x, y_supp):
    (z, pin, err, werr), _ = run(x, y_supp, trace=False)
    return (z, pin, err, werr)


# revision 7
# speedup vs baseline: 1.8619x; 1.8619x over previous
"""Trainium2 Bass kernel for the regularized newsvendor layer.

Per batch row p (weights over sorted support y), find the root z* of
  f(z) = sum_i p_i*[(y_i<=z) - CF] + g(z),
  g(z) = GAMMA * sqrt(B) * t / sqrt(t^2 + E0),  t = z - A/B,
  S = sum p, B = sum p^2, A = sum p^2 y, C = sum p^2 y^2,
  E0 = (C - A^2/B + 1e-12)/B,
then emit (z, pinball(y-z), y-z, p*(y-z)).

Instead of the reference's 60-step bisection (60 full passes over the
support), f's zero is located directly: it either sits exactly on a support
point y[k*] (the step part crosses 0 there) or between support points where
the smooth part solves in closed form:
  z = A/B - u*sqrt(E0/(GAMMA^2*B - u^2)),  u = c_{k*-1} - CF*S.
Both cases collapse to z = clamp(z_closed, 0, min(y[k*], 1)).
k* is found hierarchically: prefix sums at 16 chunk boundaries (one chunked
reduce + a 16-wide hardware scan), then an indirect-DMA gather of the one
128-wide chunk containing the crossing, resolved exactly with a scan seeded
by the chunk prefix.

Engine notes: this environment cannot run ScalarEngine activations or the
custom TensorTensorReduce (both wedge the exec unit), so everything is DVE /
GPSIMD / PE / DMA only; rsqrt and sqrt use the int32 bit-trick seed plus
Newton steps on the vector engine.

Sharding: pure data parallel across 8 cores, 512 batch rows each; y
replicated. Full inputs in, full outputs out.
"""

import sys
import functools

import numpy as np

sys.path.insert(0, "/opt/trn_rl_repo")

import concourse.bass as bass
import concourse.bacc as bacc
import concourse.mybir as mybir
import concourse.tile as tile
from concourse.bass_utils import run_bass_kernel_spmd

CF = 0.7
GAMMA = 0.1
N = 2048
BATCH = 4096
NCORES = 8
R = BATCH // NCORES   # rows per core = 512
P = 128
NT = R // P           # tiles per core = 4
NCH = 16              # chunks per row
L = N // NCH          # chunk length = 128

F32 = mybir.dt.float32
I32 = mybir.dt.int32
OP = mybir.AluOpType
AF = mybir.ActivationFunctionType
MAGIC = 0x5F3759DF


def build_nc(rows=R):
    nt = rows // P
    w = nt * NCH      # packed chunk width (64)
    wl = nt * L       # packed window width (512)
    nc = bacc.Bacc(None, target_bir_lowering=False)

    x_d = nc.dram_tensor("x", [rows, N], F32, kind="ExternalInput")
    y_d = nc.dram_tensor("y", [N], F32, kind="ExternalInput")
    z_d = nc.dram_tensor("z", [rows, 1], F32, kind="ExternalOutput")
    err_d = nc.dram_tensor("err", [rows, N], F32, kind="ExternalOutput")
    pin_d = nc.dram_tensor("pin", [rows, N], F32, kind="ExternalOutput")
    werr_d = nc.dram_tensor("werr", [rows, N], F32, kind="ExternalOutput")

    x_chunks = x_d.rearrange("r (c l) -> (r c) l", l=L)   # [rows*16, 128]
    y_chunks = y_d.rearrange("(c l) -> c l", l=L)         # [16, 128]

    def bc(t4, inner):
        # [P, nt] per-tile scalars -> broadcast view [P, nt, inner]
        return t4[:, :, None].broadcast_to([P, nt, inner])

    with tile.TileContext(nc) as tc:
        with (
            tc.tile_pool(name="const", bufs=1) as cpool,
            tc.tile_pool(name="state", bufs=1) as stp,
            tc.tile_pool(name="xp", bufs=nt) as xpool,
            tc.tile_pool(name="big", bufs=2) as bpool,
            tc.tile_pool(name="sml", bufs=2) as spool,
        ):
            # ---------- constants ----------
            yb = cpool.tile([P, N], F32)
            nc.sync.dma_start(yb[:], y_d[None, :].broadcast_to([P, N]))
            c15 = cpool.tile([P, wl], F32)
            nc.vector.memset(c15[:], 1.5)
            # gather base offsets: idx0[p, t] = 16*p + 2048*t (int32 -> f32)
            pfi = cpool.tile([P, nt], I32)
            nc.gpsimd.iota(pfi[:], pattern=[[P * NCH, nt]], base=0,
                           channel_multiplier=NCH)
            pft = cpool.tile([P, nt], F32)
            nc.vector.tensor_copy(pft[:], pfi[:])

            def newton(rt, s2t, iters, width):
                # rt <- rsqrt(s2t) via bit-trick seed + Newton iterations
                ii = rt[:].bitcast(I32)
                nc.vector.tensor_scalar(ii, s2t[:].bitcast(I32), 1, None,
                                        op0=OP.logical_shift_right)
                nc.vector.tensor_scalar(ii, ii, -1, None, op0=OP.mult)
                nc.vector.tensor_scalar(ii, ii, MAGIC, None, op0=OP.add)
                r2 = spool.tile([P, width], F32, tag=f"nw_r2_{width}")
                a = spool.tile([P, width], F32, tag=f"nw_a_{width}")
                for _ in range(iters):
                    nc.vector.tensor_tensor(r2[:], rt[:], rt[:], op=OP.mult)
                    nc.vector.tensor_tensor(a[:], s2t[:], r2[:], op=OP.mult)
                    nc.vector.scalar_tensor_tensor(a[:], a[:], -0.5,
                                                   c15[:, 0:width],
                                                   op0=OP.mult, op1=OP.add)
                    nc.vector.tensor_tensor(rt[:], rt[:], a[:], op=OP.mult)

            # ---------- packed state ----------
            B4 = stp.tile([P, nt], F32)
            A4 = stp.tile([P, nt], F32)
            C4 = stp.tile([P, nt], F32)
            cs64 = stp.tile([P, w], F32)
            cb64 = stp.tile([P, w], F32)
            xg = stp.tile([P, wl], F32)
            yg = stp.tile([P, wl], F32)

            xts = []
            # ---------- phase 1: per-tile streams ----------
            for t in range(nt):
                r0 = t * P
                xt = xpool.tile([P, N], F32, tag="xt")
                xts.append(xt)
                nc.sync.dma_start(xt[:], x_d[r0:r0 + P, :])

                q = bpool.tile([P, N], F32, tag="scr")
                nc.vector.scalar_tensor_tensor(q[:], xt[:], 0.0, xt[:],
                                               op0=OP.bypass, op1=OP.mult,
                                               accum_out=B4[:, t:t + 1])
                qy = bpool.tile([P, N], F32, tag="scr")
                nc.vector.scalar_tensor_tensor(qy[:], q[:], 0.0, yb[:],
                                               op0=OP.bypass, op1=OP.mult,
                                               accum_out=A4[:, t:t + 1])
                qy2 = bpool.tile([P, N], F32, tag="scr")
                nc.vector.scalar_tensor_tensor(qy2[:], qy[:], 0.0, yb[:],
                                               op0=OP.bypass, op1=OP.mult,
                                               accum_out=C4[:, t:t + 1])

                nc.vector.tensor_reduce(
                    cs64[:, t * NCH:(t + 1) * NCH],
                    xt.rearrange("p (c l) -> p c l", c=NCH),
                    axis=mybir.AxisListType.X, op=OP.add)
                nc.vector.tensor_tensor_scan(
                    cb64[:, t * NCH:(t + 1) * NCH],
                    cs64[:, t * NCH:(t + 1) * NCH],
                    cs64[:, t * NCH:(t + 1) * NCH],
                    0.0, op0=OP.add, op1=OP.bypass)

            # ---------- phase 2: packed solve ----------
            S4 = cb64[:, NCH - 1::NCH]  # [P, nt] view: per-tile total sums

            Bg = stp.tile([P, nt], F32)
            nc.vector.tensor_scalar(Bg[:], B4[:], 1e-20, None, op0=OP.max)
            invB = stp.tile([P, nt], F32)
            nc.vector.reciprocal(invB[:], Bg[:])
            AB = stp.tile([P, nt], F32)
            nc.vector.tensor_tensor(AB[:], A4[:], invB[:], op=OP.mult)
            CB = spool.tile([P, nt], F32, tag="CB")
            nc.vector.tensor_tensor(CB[:], C4[:], invB[:], op=OP.mult)
            ABsq = spool.tile([P, nt], F32, tag="ABsq")
            nc.vector.tensor_tensor(ABsq[:], AB[:], AB[:], op=OP.mult)
            E0 = stp.tile([P, nt], F32)
            nc.vector.tensor_tensor(E0[:], CB[:], ABsq[:], op=OP.subtract)
            eps12 = spool.tile([P, nt], F32, tag="eps12")
            nc.vector.tensor_scalar(eps12[:], invB[:], 1e-12, None, op0=OP.mult)
            nc.vector.tensor_tensor(E0[:], E0[:], eps12[:], op=OP.add)
            nc.vector.tensor_scalar(E0[:], E0[:], 1e-30, None, op0=OP.max)
            CFS = stp.tile([P, nt], F32)
            nc.vector.tensor_scalar(CFS[:], S4, CF, None, op0=OP.mult)
            G2B = stp.tile([P, nt], F32)
            nc.vector.tensor_scalar(G2B[:], Bg[:], GAMMA * GAMMA, None,
                                    op0=OP.mult)
            rsB = spool.tile([P, nt], F32, tag="rsB")
            newton(rsB, Bg, 2, nt)
            GB = stp.tile([P, nt], F32)
            nc.vector.tensor_tensor(GB[:], Bg[:], rsB[:], op=OP.mult)
            nc.vector.tensor_scalar(GB[:], GB[:], GAMMA, None, op0=OP.mult)

            # chunk-boundary f: find crossing chunk j* per (row, tile)
            ybnd = yb[:, L - 1::L][:, None, :].broadcast_to([P, nt, NCH])
            t64 = spool.tile([P, w], F32, tag="t64")
            t64v = t64.rearrange("p (t c) -> p t c", t=nt)
            nc.vector.tensor_tensor(t64v, ybnd, bc(AB, NCH), op=OP.subtract)
            s64 = spool.tile([P, w], F32, tag="s64")
            nc.vector.tensor_tensor(s64[:], t64[:], t64[:], op=OP.mult)
            nc.vector.tensor_tensor(s64.rearrange("p (t c) -> p t c", t=nt),
                                    s64.rearrange("p (t c) -> p t c", t=nt),
                                    bc(E0, NCH), op=OP.add)
            rh64 = spool.tile([P, w], F32, tag="rh64")
            newton(rh64, s64, 2, w)
            m64 = spool.tile([P, w], F32, tag="m64")
            nc.vector.tensor_tensor(m64[:], t64[:], rh64[:], op=OP.mult)
            nc.vector.tensor_tensor(m64.rearrange("p (t c) -> p t c", t=nt),
                                    m64.rearrange("p (t c) -> p t c", t=nt),
                                    bc(GB, NCH), op=OP.mult)
            h64 = spool.tile([P, w], F32, tag="h64")
            nc.vector.tensor_tensor(h64[:], m64[:], cb64[:], op=OP.add)
            mask64 = spool.tile([P, w], F32, tag="mask64")
            nc.vector.tensor_tensor(mask64.rearrange("p (t c) -> p t c", t=nt),
                                    h64.rearrange("p (t c) -> p t c", t=nt),
                                    bc(CFS, NCH), op=OP.is_lt)
            jf = stp.tile([P, nt], F32)
            nc.vector.tensor_reduce(jf[:],
                                    mask64.rearrange("p (t c) -> p t c", t=nt),
                                    axis=mybir.AxisListType.X, op=OP.add)
            cm64 = spool.tile([P, w], F32, tag="cm64")
            nc.vector.tensor_tensor(cm64[:], cb64[:], mask64[:], op=OP.mult)
            cpch = stp.tile([P, nt], F32)
            nc.vector.tensor_reduce(cpch[:],
                                    cm64.rearrange("p (t c) -> p t c", t=nt),
                                    axis=mybir.AxisListType.X, op=OP.max)

            # gather the crossing chunk of x and y per (row, tile)
            jmin = stp.tile([P, nt], F32)
            nc.vector.tensor_scalar(jmin[:], jf[:], float(NCH - 1), None,
                                    op0=OP.min)
            gxf = spool.tile([P, nt], F32, tag="gxf")
            nc.vector.tensor_tensor(gxf[:], pft[:], jmin[:], op=OP.add)
            gxi = stp.tile([P, nt], I32)
            nc.vector.tensor_copy(gxi[:], gxf[:])
            gyi = stp.tile([P, nt], I32)
            nc.vector.tensor_copy(gyi[:], jmin[:])
            for t in range(nt):
                nc.gpsimd.indirect_dma_start(
                    xg[:, t * L:(t + 1) * L], None, x_chunks,
                    bass.IndirectOffsetOnAxis(ap=gxi[:, t:t + 1], axis=0))
                nc.gpsimd.indirect_dma_start(
                    yg[:, t * L:(t + 1) * L], None, y_chunks,
                    bass.IndirectOffsetOnAxis(ap=gyi[:, t:t + 1], axis=0))

            # within-chunk exact resolution (packed [P, 512])
            cloc = stp.tile([P, wl], F32)
            for t in range(nt):
                nc.vector.tensor_tensor_scan(
                    cloc[:, t * L:(t + 1) * L], xg[:, t * L:(t + 1) * L],
                    xg[:, t * L:(t + 1) * L], cpch[:, t:t + 1],
                    op0=OP.add, op1=OP.bypass)
            tw = spool.tile([P, wl], F32, tag="tw")
            nc.vector.tensor_tensor(tw.rearrange("p (t c) -> p t c", t=nt),
                                    yg.rearrange("p (t c) -> p t c", t=nt),
                                    bc(AB, L), op=OP.subtract)
            s2w = spool.tile([P, wl], F32, tag="s2w")
            nc.vector.tensor_tensor(s2w[:], tw[:], tw[:], op=OP.mult)
            nc.vector.tensor_tensor(s2w.rearrange("p (t c) -> p t c", t=nt),
                                    s2w.rearrange("p (t c) -> p t c", t=nt),
                                    bc(E0, L), op=OP.add)
            rhw = spool.tile([P, wl], F32, tag="rhw")
            newton(rhw, s2w, 2, wl)
            mw = spool.tile([P, wl], F32, tag="mw")
            nc.vector.tensor_tensor(mw[:], tw[:], rhw[:], op=OP.mult)
            nc.vector.tensor_tensor(mw.rearrange("p (t c) -> p t c", t=nt),
                                    mw.rearrange("p (t c) -> p t c", t=nt),
                                    bc(GB, L), op=OP.mult)
            hw = spool.tile([P, wl], F32, tag="hw")
            nc.vector.tensor_tensor(hw[:], mw[:], cloc[:], op=OP.add)
            maskw = spool.tile([P, wl], F32, tag="maskw")
            nc.vector.tensor_tensor(maskw.rearrange("p (t c) -> p t c", t=nt),
                                    hw.rearrange("p (t c) -> p t c", t=nt),
                                    bc(CFS, L), op=OP.is_lt)
            cmw = spool.tile([P, wl], F32, tag="cmw")
            nc.vector.tensor_tensor(cmw[:], cloc[:], maskw[:], op=OP.mult)
            cprev = stp.tile([P, nt], F32)
            nc.vector.tensor_reduce(cprev[:],
                                    cmw.rearrange("p (t c) -> p t c", t=nt),
                                    axis=mybir.AxisListType.X, op=OP.max)
            nc.vector.tensor_tensor(cprev[:], cprev[:], cpch[:], op=OP.max)
            ymw = spool.tile([P, wl], F32, tag="ymw")
            nc.vector.tensor_tensor(ymw[:], yg[:], maskw[:], op=OP.add)
            ystar = stp.tile([P, nt], F32)
            nc.vector.tensor_reduce(ystar[:],
                                    ymw.rearrange("p (t c) -> p t c", t=nt),
                                    axis=mybir.AxisListType.X, op=OP.min)

            # closed form + clamp -> z
            u = stp.tile([P, nt], F32)
            nc.vector.tensor_tensor(u[:], cprev[:], CFS[:], op=OP.subtract)
            u2 = spool.tile([P, nt], F32, tag="u2")
            nc.vector.tensor_tensor(u2[:], u[:], u[:], op=OP.mult)
            den = spool.tile([P, nt], F32, tag="den")
            nc.vector.tensor_tensor(den[:], G2B[:], u2[:], op=OP.subtract)
            nc.vector.tensor_scalar(den[:], den[:], 1e-20, None, op0=OP.max)
            rden = spool.tile([P, nt], F32, tag="rden")
            nc.vector.reciprocal(rden[:], den[:])
            rat = spool.tile([P, nt], F32, tag="rat")
            nc.vector.tensor_tensor(rat[:], E0[:], rden[:], op=OP.mult)
            rsr = spool.tile([P, nt], F32, tag="rsr")
            newton(rsr, rat, 3, nt)
            s1 = spool.tile([P, nt], F32, tag="s1")
            nc.vector.tensor_tensor(s1[:], rat[:], rsr[:], op=OP.mult)
            us = spool.tile([P, nt], F32, tag="us")
            nc.vector.tensor_tensor(us[:], u[:], s1[:], op=OP.mult)
            z4 = stp.tile([P, nt], F32)
            nc.vector.tensor_tensor(z4[:], AB[:], us[:], op=OP.subtract)
            yc = spool.tile([P, nt], F32, tag="yc")
            nc.vector.tensor_scalar(yc[:], ystar[:], 1.0, None, op0=OP.min)
            nc.vector.tensor_tensor(z4[:], z4[:], yc[:], op=OP.min)
            nc.vector.tensor_scalar(z4[:], z4[:], 0.0, None, op0=OP.max)
            nc.sync.dma_start(
                z_d.rearrange("(t p) o -> p (t o)", p=P), z4[:])

            # ---------- phase 3: big outputs ----------
            for t in range(nt):
                r0 = t * P
                xt = xts[t]
                errt = bpool.tile([P, N], F32, tag="errt")
                nc.vector.tensor_scalar(errt[:], yb[:], z4[:, t:t + 1], None,
                                        op0=OP.subtract)
                nc.sync.dma_start(err_d[r0:r0 + P, :], errt[:])
                a7 = bpool.tile([P, N], F32, tag="a7")
                nc.vector.tensor_scalar(a7[:], errt[:], CF, None, op0=OP.mult)
                pin = bpool.tile([P, N], F32, tag="pin")
                nc.vector.scalar_tensor_tensor(pin[:], a7[:], (CF - 1.0) / CF,
                                               a7[:], op0=OP.mult, op1=OP.max)
                nc.sync.dma_start(pin_d[r0:r0 + P, :], pin[:])
                werrt = bpool.tile([P, N], F32, tag="werrt")
                nc.vector.tensor_tensor(werrt[:], xt[:], errt[:], op=OP.mult)
                nc.sync.dma_start(werr_d[r0:r0 + P, :], werrt[:])

    nc.finalize()
    return nc


@functools.lru_cache(maxsize=1)
def _cached_nc():
    return build_nc(R)


def run(x, y_supp, trace=False):
    x = np.ascontiguousarray(x, dtype=np.float32)
    y = np.ascontiguousarray(y_supp, dtype=np.float32)
    assert x.shape == (BATCH, N) and y.shape == (N,)
    nc = _cached_nc()
    in_maps = [{"x": x[i * R:(i + 1) * R], "y": y} for i in range(NCORES)]
    res = run_bass_kernel_spmd(nc, in_maps, core_ids=list(range(NCORES)),
                               trace=trace)
    z = np.concatenate([r["z"] for r in res.results], axis=0)
    err = np.concatenate([r["err"] for r in res.results], axis=0)
    pin = np.concatenate([r["pin"] for r in res.results], axis=0)
    werr = np.concatenate([r["werr"] for r in res.results], axis=0)
    return (z, pin, err, werr), res


def kernel(x, y_supp):
    (z, pin, err, werr), _ = run(x, y_supp, trace=False)
    return (z, pin, err, werr)
